# revision 22
# baseline (speedup 1.0000x reference)
"""Trainium2 Bass kernel: causal multi-head self-attention with RoPE.

Problem: x[2,2048,1024], 16 heads, d_k=64, causal, RoPE(theta=1e4),
out = (softmax(rope(Q)rope(K)^T/8) V) WO^T.

Sharding (8 cores): data-parallel over batch (2) x head-parallel over
head groups (4 heads per core).  Each core computes Q/K/V projections
for its 4 heads, flash-style causal attention, and a partial output
projection over its 256 channels; the host sums the 4 partials per
batch element.

v3 structure:
  - scores for the 2 heads of a pair issue as adjacent K=64 matmuls at
    row groups 0/64 into different PSUM banks -> concurrent on the PE.
  - causal masking post-exp on GpSimd (0/1 tri multiply on the 128-wide
    diagonal sub-block) - no eye/mask matmuls on TensorE at all; score
    matmuls and exp are trimmed to cols >= c0 (PV skips them anyway).
  - Q/K projection and output projection reuse the stationary operand:
    explicit ldweights + two ldweights=False matmuls into alternating
    PSUM banks (walrus is compiled with ldw-opt off, so every
    self-loading matmul pays an exposed LDWEIGHTS + drain ~2x cost).
  - j-outer pipeline: attention(pair,j) for both pairs, then per-j
    normalization + output projection; projection/RoPE emitted in
    slices between attention tiles so the Tile scheduler can fill
    TensorE stalls and the PE never re-throttles (HAM).
  - PSUM->SBUF evacuations split across VectorE and ScalarE by phase
    (ScalarE takes the ones in its exp-idle windows).
Device layouts as baseline: xt pre-chunked [4,128,8,512]; Qt/Kt rows
[32 even,32 odd] per head (host permutes W columns) so RoPE is pure
row-block ops; V [128,16,4,65] with a ones 65th column producing
softmax denominators inside the PV matmul.
"""

import os
import sys

for _p in ("/opt/trn_rl_repo",):
    if _p not in sys.path:
        sys.path.insert(0, _p)

import numpy as np
import ml_dtypes

BF16 = ml_dtypes.bfloat16

D = 1024
S = 2048
H = 16
DK = 64
HPC = 4          # heads per core
NCORES = 8
THETA = 10000.0

_COMPILED = {}


def _build_nc():
    import concourse.bass as bass  # noqa: F401
    import concourse.bacc as bacc
    import concourse.mybir as mybir
    import concourse.tile as tile

    bf16 = mybir.dt.bfloat16
    f32 = mybir.dt.float32
    Exp = mybir.ActivationFunctionType.Exp

    nc = bacc.Bacc(
        "TRN2", target_bir_lowering=False, debug=False, num_devices=NCORES
    )
    xt_d = nc.declare_dram_parameter("xt", [4, 128, 8, 512], bf16, isOutput=False)
    wq_d = nc.declare_dram_parameter("wq", [128, 8, 256], bf16, isOutput=False)
    wk_d = nc.declare_dram_parameter("wk", [128, 8, 256], bf16, isOutput=False)
    wv_d = nc.declare_dram_parameter("wv", [128, 8, 256], bf16, isOutput=False)
    wo_d = nc.declare_dram_parameter("wo", [128, 2, D], bf16, isOutput=False)
    cos_d = nc.declare_dram_parameter("cosb", [32, S], bf16, isOutput=False)
    sin_d = nc.declare_dram_parameter("sinb", [2, 32, S], bf16, isOutput=False)
    msk_d = nc.declare_dram_parameter("msk", [128, 128], bf16, isOutput=False)
    ind_d = nc.declare_dram_parameter("ind", [8, 4, 128], bf16, isOutput=False)
    out_d = nc.declare_dram_parameter("out", [S, D], bf16, isOutput=True)

    with tile.TileContext(nc) as tc:
        with tc.tile_pool(name="const", bufs=1) as const:
            x_sb = const.tile([128, 8, S], bf16)
            wq_sb = const.tile([128, 8, 256], bf16)
            wk_sb = const.tile([128, 8, 256], bf16)
            wv_sb = const.tile([128, 8, 256], bf16)
            wo_sb = const.tile([128, 2, D], bf16)
            cos_sb = const.tile([128, S], bf16)
            sin_sb = const.tile([128, S], bf16)
            msk_sb = const.tile([128, 128], bf16)
            ind_sb = const.tile([8, 4, 128], bf16)
            v_sb = const.tile([128, 16, 4, 65], bf16)
            qraw = [const.tile([128, S], bf16, name=f"qraw{i}") for i in range(2)]
            kraw = [const.tile([128, S], bf16, name=f"kraw{i}") for i in range(2)]
            qrot = [const.tile([128, S], bf16, name=f"qrot{i}") for i in range(2)]
            krot = [const.tile([128, S], bf16, name=f"krot{i}") for i in range(2)]
            at = [const.tile([128, S], bf16, name=f"at{i}") for i in range(2)]
            atn = [const.tile([128, 4, 512], bf16, name=f"atn{i}")
                   for i in range(2)]
            # den rows j*2+hl; the two head-pairs live in column halves so
            # every engine op stays at partition base 0 (the custom-DVE
            # reciprocal misbehaves at non-zero partition bases)
            den_sb = const.tile([8, 1024], bf16)
            denf = const.tile([8, 1024], f32)
            rc = const.tile([8, 1024], f32)
            rcb = const.tile([8, 1024], bf16)

            # sync (HW DGE): startup-critical bytes in first-use order.
            # cos/sin ship deduplicated ([32,S] blocks) and replicate
            # on-device - input DMA bandwidth bounds the kernel start.
            for cp in range(4):
                c2 = slice(cp * 2, cp * 2 + 2)
                nc.sync.dma_start(wk_sb[:, c2, :], wk_d[:, c2, :])
                nc.sync.dma_start(x_sb[:, c2, 0:512], xt_d[0][:, c2, :])
            nc.sync.dma_start(wq_sb[:], wq_d[:])
            nc.sync.dma_start(x_sb[:, :, 512:1024], xt_d[1])
            nc.gpsimd.dma_start(cos_sb[0:32, :], cos_d[:])
            nc.gpsimd.dma_start(sin_sb[0:32, :], sin_d[0])
            nc.gpsimd.dma_start(sin_sb[32:64, :], sin_d[1])
            for blk in range(1, 4):
                nc.gpsimd.dma_start(
                    cos_sb[blk * 32:(blk + 1) * 32, :], cos_sb[0:32, :]
                )
            nc.gpsimd.dma_start(sin_sb[64:96, :], sin_sb[0:32, :])
            nc.gpsimd.dma_start(sin_sb[96:128, :], sin_sb[32:64, :])
            nc.gpsimd.dma_start(wv_sb[:], wv_d[:])
            nc.vector.memset(v_sb[:, :, :, 64:65], 1.0)
            # den rows for not-yet-computed (pair, j) read as 1.0 by the
            # full-tile reciprocal in norm()
            nc.vector.memset(den_sb[:], 1.0)

            with tc.tile_pool(name="gen", bufs=2, space="PSUM") as gen, \
                 tc.tile_pool(name="scp", bufs=2, space="PSUM") as scp, \
                 tc.tile_pool(name="pop", bufs=2, space="PSUM") as pop, \
                 tc.tile_pool(name="ptp", bufs=4) as ptp, \
                 tc.tile_pool(name="stg", bufs=4) as stg:

                def proj_qk(ot, nsls, w_sb, raw, cast_eng):
                    # paired-nsl with stationary-weight reuse: one explicit
                    # ldweights serves two matmuls into alternating banks
                    na, nb = nsls
                    pa = gen.tile([128, 512], f32, tag="gen", name="pja")
                    pb = gen.tile([128, 512], f32, tag="gen", name="pjb")
                    for c in range(8):
                        w_ap = w_sb[:, c, ot * 128:(ot + 1) * 128]
                        nc.tensor.matmul(
                            pa[:], w_ap,
                            x_sb[:, c, na * 512:(na + 1) * 512],
                            start=(c == 0), stop=(c == 7),
                        )
                        nc.tensor.matmul(
                            pb[:], w_ap,
                            x_sb[:, c, nb * 512:(nb + 1) * 512],
                            start=(c == 0), stop=(c == 7),
                        )
                    for ps, nsl in ((pa, na), (pb, nb)):
                        if cast_eng == "scalar":
                            nc.scalar.copy(
                                raw[ot][:, nsl * 512:(nsl + 1) * 512], ps[:]
                            )
                        else:
                            nc.vector.tensor_copy(
                                raw[ot][:, nsl * 512:(nsl + 1) * 512], ps[:]
                            )

                def rope_half(raw, rot, h):
                    cl = slice(h * 1024, (h + 1) * 1024)
                    sw = stg.tile([128, 1024], bf16, tag="sw", name="sw")
                    t1 = stg.tile([128, 1024], bf16, tag="t1", name="t1")
                    for blk in range(4):
                        src = blk ^ 1
                        nc.sync.dma_start(
                            sw[blk * 32:(blk + 1) * 32, :],
                            raw[src * 32:(src + 1) * 32, cl],
                        )
                    nc.vector.tensor_mul(t1[:], raw[:, cl], cos_sb[:, cl])
                    nc.vector.tensor_mul(sw[:], sw[:], sin_sb[:, cl])
                    nc.vector.tensor_add(rot[:, cl], t1[:], sw[:])

                def proj_v(sbs, cast_eng):
                    # paired s-blocks into alternating PSUM banks so matmul
                    # drains hide under the partner's fill
                    for sb0 in list(sbs)[::2]:
                        pa = gen.tile([128, 512], f32, tag="gen", name="pva")
                        pb = gen.tile([128, 512], f32, tag="gen", name="pvb")
                        for c in range(8):
                            nc.tensor.matmul(
                                pa[:, 0:256],
                                x_sb[:, c, sb0 * 128:(sb0 + 1) * 128],
                                wv_sb[:, c, :],
                                start=(c == 0), stop=(c == 7),
                            )
                            nc.tensor.matmul(
                                pb[:, 0:256],
                                x_sb[:, c, (sb0 + 1) * 128:(sb0 + 2) * 128],
                                wv_sb[:, c, :],
                                start=(c == 0), stop=(c == 7),
                            )
                        for sb, ps in ((sb0, pa), (sb0 + 1, pb)):
                            src = ps[:, 0:256].rearrange("p (h d) -> p h d", h=4)
                            if cast_eng == "scalar":
                                nc.scalar.copy(v_sb[:, sb, :, 0:64], src)
                            else:
                                nc.vector.tensor_copy(v_sb[:, sb, :, 0:64], src)

                def attn(ot, j):
                    qr, kr = qrot[ot], krot[ot]
                    nkb = 4 * (j + 1)
                    hA, hB = 2 * ot, 2 * ot + 1
                    poA = pop.tile([65, 512], f32, tag="po", name="poA")
                    poB = pop.tile([65, 512], f32, tag="po", name="poB")
                    for kb in range(nkb):
                        dg = kb - 4 * j
                        c0 = dg * 128 if dg > 0 else 0
                        sp = scp.tile([128, 1024], f32, tag="sc", name="sp")
                        pt = ptp.tile([128, 1024], bf16, tag="pt", name="pt")
                        # 2-head packed score matmuls (K=64 row groups 0/64)
                        nc.tensor.matmul(
                            sp[:, c0:512],
                            kr[0:64, kb * 128:(kb + 1) * 128],
                            qr[0:64, j * 512 + c0:(j + 1) * 512],
                            start=True, stop=True,
                        )
                        nc.tensor.matmul(
                            sp[:, 512 + c0:1024],
                            kr[64:128, kb * 128:(kb + 1) * 128],
                            qr[64:128, j * 512 + c0:(j + 1) * 512],
                            start=True, stop=True,
                        )
                        nc.scalar.activation(
                            pt[:, c0:1024], sp[:, c0:1024], Exp, scale=0.125
                        )
                        if dg >= 0:
                            # causal mask: zero the exp'd upper-tri of the
                            # 128-wide diagonal sub-block on GpSimd
                            nc.gpsimd.tensor_mul(
                                pt[:, c0:c0 + 128], pt[:, c0:c0 + 128],
                                msk_sb[:],
                            )
                            nc.gpsimd.tensor_mul(
                                pt[:, 512 + c0:512 + c0 + 128],
                                pt[:, 512 + c0:512 + c0 + 128],
                                msk_sb[:],
                            )
                        nc.tensor.matmul(
                            poA[:, c0:512],
                            v_sb[:, kb, hA, 0:65],
                            pt[:, c0:512],
                            start=(kb == 0), stop=(kb == nkb - 1),
                        )
                        nc.tensor.matmul(
                            poB[:, c0:512],
                            v_sb[:, kb, hB, 0:65],
                            pt[:, 512 + c0:1024],
                            start=(kb == 0), stop=(kb == nkb - 1),
                        )
                    for hl, po in ((0, poA), (1, poB)):
                        tm = stg.tile([65, 512], bf16, tag="tm", name="tm")
                        nc.vector.tensor_copy(tm[:], po[:])
                        r0 = hl * 64
                        nc.sync.dma_start(
                            at[ot][r0:r0 + 64, j * 512:(j + 1) * 512],
                            tm[0:64, :],
                        )
                        dr = j * 2 + hl
                        nc.sync.dma_start(
                            den_sb[dr:dr + 1, ot * 512:(ot + 1) * 512],
                            tm[64:65, :],
                        )

                def norm_ot(ot, j):
                    # per-pair normalization so it can overlap the other
                    # pair's attention; pairs occupy column halves so all
                    # ops run at partition base 0
                    c0 = ot * 512
                    nc.vector.tensor_copy(
                        denf[:, c0:c0 + 512], den_sb[:, c0:c0 + 512]
                    )
                    nc.vector.reciprocal_approx_fast(
                        rc[:, c0:c0 + 512], denf[:, c0:c0 + 512]
                    )
                    nc.vector.tensor_copy(
                        rcb[:, c0:c0 + 512], rc[:, c0:c0 + 512]
                    )
                    rbp = gen.tile([128, 512], f32, tag="gen", name="rb")
                    nc.tensor.matmul(
                        rbp[:], ind_sb[:, j, :],
                        rcb[:, c0:c0 + 512], start=True, stop=True,
                    )
                    nc.vector.tensor_mul(
                        atn[ot][:, j, :],
                        at[ot][:, j * 512:(j + 1) * 512],
                        rbp[:],
                    )

                def outproj(j, cast_eng="vector"):
                    for sbi in range(4):
                        sb = j * 4 + sbi
                        pf0 = gen.tile([128, 512], f32, tag="gen", name="pf0")
                        pf1 = gen.tile([128, 512], f32, tag="gen", name="pf1")
                        for ich in range(2):
                            a_ap = atn[ich][:, j, sbi * 128:(sbi + 1) * 128]
                            nc.tensor.matmul(
                                pf0[:], a_ap, wo_sb[:, ich, 0:512],
                                start=(ich == 0), stop=(ich == 1),
                            )
                            nc.tensor.matmul(
                                pf1[:], a_ap, wo_sb[:, ich, 512:1024],
                                start=(ich == 0), stop=(ich == 1),
                            )
                        for osl, pf in ((0, pf0), (1, pf1)):
                            ob = stg.tile([128, 512], bf16, tag="ob", name="ob")
                            if cast_eng == "scalar":
                                nc.scalar.copy(ob[:], pf[:])
                            else:
                                nc.vector.tensor_copy(ob[:], pf[:])
                            nc.sync.dma_start(
                                out_d[sb * 128:(sb + 1) * 128,
                                      osl * 512:(osl + 1) * 512],
                                ob[:],
                            )

                # ---- emission order == scheduler priority ----
                # fillers (later-needed projections/ropes/norms/outproj)
                # go in a low-priority band so the scheduler only runs
                # them when the attention chain (which feeds ScalarE's
                # exps) has nothing ready
                _fb = [1_000_000]

                def low(fn, *a, **k):
                    saved = tc.cur_priority
                    tc.cur_priority = _fb[0]
                    fn(*a, **k)
                    _fb[0] = tc.cur_priority
                    tc.cur_priority = saved

                proj_qk(0, (0, 1), wk_sb, kraw, "vector")
                rope_half(kraw[0], krot[0], 0)
                proj_qk(0, (0, 1), wq_sb, qraw, "vector")
                rope_half(qraw[0], qrot[0], 0)
                proj_qk(1, (0, 1), wk_sb, kraw, "vector")
                rope_half(kraw[1], krot[1], 0)
                proj_qk(1, (0, 1), wq_sb, qraw, "vector")
                rope_half(qraw[1], qrot[1], 0)
                nc.sync.dma_start(msk_sb[:], msk_d[:])
                # later-needed inputs: low band, after the startup set
                low(nc.sync.dma_start, x_sb[:, :, 1024:1536], xt_d[2])
                low(nc.sync.dma_start, x_sb[:, :, 1536:2048], xt_d[3])
                low(nc.sync.dma_start, wo_sb[:], wo_d[:])
                low(nc.sync.dma_start, ind_sb[:], ind_d[:])
                proj_v(range(0, 8), "vector")
                attn(0, 0)
                low(proj_qk, 0, (2, 3), wk_sb, kraw, "vector")
                low(rope_half, kraw[0], krot[0], 1)
                attn(1, 0)
                low(proj_qk, 0, (2, 3), wq_sb, qraw, "vector")
                low(rope_half, qraw[0], qrot[0], 1)
                low(norm_ot, 0, 0)
                low(norm_ot, 1, 0)
                attn(0, 1)
                low(proj_qk, 1, (2, 3), wk_sb, kraw, "vector")
                low(rope_half, kraw[1], krot[1], 1)
                attn(1, 1)
                low(proj_qk, 1, (2, 3), wq_sb, qraw, "vector")
                low(rope_half, qraw[1], qrot[1], 1)
                low(proj_v, range(8, 16), "vector")
                low(norm_ot, 0, 1)
                low(norm_ot, 1, 1)
                low(outproj, 0)
                attn(0, 2)
                attn(1, 2)
                low(norm_ot, 0, 2)
                low(norm_ot, 1, 2)
                low(outproj, 1)
                attn(0, 3)
                low(norm_ot, 0, 3)
                attn(1, 3)
                norm_ot(1, 3)
                low(outproj, 2)
                outproj(3, "scalar")
    nc.compile()
    return nc


def _host_prep(x, token_positions, WQ, WK, WV, WO):
    """Build the 8 per-core input maps."""
    pos = np.asarray(token_positions).astype(np.float32)
    k = np.arange(DK // 2, dtype=np.float32)
    inv_freq = 1.0 / (THETA ** (2.0 * k / DK))
    ang = pos[:, None] * inv_freq[None, :]          # [S, 32]
    c32 = np.cos(ang).T.astype(np.float32)          # [32, S]
    s32 = np.sin(ang).T.astype(np.float32)
    cosb = c32.astype(BF16)                          # [32, S]
    sinb = np.stack([-s32, s32], axis=0).astype(BF16)  # [2, 32, S]
    # 0/1 keep-mask for the 128-wide diagonal sub-block: keep key k <= query q
    kk = np.arange(128)[:, None]
    qq = np.arange(128)[None, :]
    msk = np.where(kk <= qq, 1.0, 0.0).astype(BF16)  # [128, 128]
    # indicator matrices for denominator broadcast:
    # ind[i, j, r] = 1 iff i == j*2 + (r//64)   (same for both pairs)
    ind = np.zeros((8, 4, 128), dtype=np.float32)
    for j in range(4):
        for r in range(128):
            ind[j * 2 + (r // 64), j, r] = 1.0
    ind = ind.astype(BF16)

    perm = np.concatenate([np.arange(0, DK, 2), np.arange(1, DK, 2)])  # evens,odds

    in_maps = []
    for core in range(NCORES):
        b, hg = divmod(core, 4)
        ch0 = hg * 256
        qk_rows = np.concatenate([ch0 + hl * 64 + perm for hl in range(HPC)])
        def dev_w(w):  # [D, M] -> [128, 8, M] (contraction chunks)
            return np.ascontiguousarray(
                w.reshape(8, 128, -1).transpose(1, 0, 2)
            ).astype(BF16)

        xt = np.asarray(x[b]).T                       # [D, S]
        xt4 = np.ascontiguousarray(
            xt.reshape(8, 128, 4, 512).transpose(2, 1, 0, 3)
        ).astype(BF16)                                # [4, 128, 8, 512]
        in_maps.append({
            "xt": xt4,
            "wq": dev_w(np.asarray(WQ)[qk_rows, :].T),
            "wk": dev_w(np.asarray(WK)[qk_rows, :].T),
            "wv": dev_w(np.asarray(WV)[ch0:ch0 + 256, :].T),
            "wo": np.ascontiguousarray(
                np.asarray(WO)[:, ch0:ch0 + 256].T.reshape(2, 128, D)
                .transpose(1, 0, 2)
            ).astype(BF16),
            "cosb": cosb,
            "sinb": sinb,
            "msk": msk,
            "ind": ind,
        })
    return in_maps


LAST_EXEC_NS = None
LAST_RES = None


def kernel(x, token_positions, WQ, WK, WV, WO):
    global LAST_EXEC_NS, LAST_RES
    from concourse.bass_utils import run_bass_kernel_spmd

    if "nc" not in _COMPILED:
        _COMPILED["nc"] = _build_nc()
    nc = _COMPILED["nc"]

    in_maps = _host_prep(x, token_positions, WQ, WK, WV, WO)
    res = run_bass_kernel_spmd(nc, in_maps, list(range(NCORES)))
    LAST_EXEC_NS = res.exec_time_ns
    LAST_RES = res

    out = np.zeros((2, S, D), dtype=np.float32)
    for core in range(NCORES):
        out[core // 4] += np.asarray(res.results[core]["out"], dtype=np.float32)
    return out
